# revision 4
# baseline (speedup 1.0000x reference)
"""Bass/Trainium2 kernel for batched dot-product attention.

Problem: q,k,v [B=4, S=4096, D=1024]; projections to dk=dv=128; softmax
attention per batch element.  Sharded over 8 NeuronCores as (batch,
query-half): core c handles batch c//2, queries (c%2)*2048 ... +2048.

All layouts on-chip keep the contraction dimension on SBUF partitions:
  qT/kT/vT   [d_model, seq]   (host pre-transposed, bf16)
  kpT/qpT    [dk, seq]        (projection output, bf16)
  vp         [seq, dv]        (natural layout via PE transpose, f32)
  S^T tiles  [keys, q]        (scores transposed)
  out^T      [dv, q]          (final output transposed; host undoes)

Softmax is computed without max-subtraction (scores are O(1) by
construction), denominators via a ones-vector matmul over the partition
axis.  Scale 1/sqrt(dk) is folded into wq/bq on the host.
"""

import math

import numpy as np
import ml_dtypes

import concourse.bass as bass
import concourse.tile as tile
from concourse import bacc, mybir
from concourse.bass_utils import run_bass_kernel_spmd

B, S, DM, DK, DV = 4, 4096, 1024, 128, 128
N_CORES = 8
SQ = S // 2          # queries per core
NQB = SQ // 512      # query blocks of 512 per core (4)
NKC = S // 128       # key chunks of 128 (32)
NMC = DM // 128      # d_model chunks (8)
NKB = S // 512       # key blocks of 512 (8)

BF16 = mybir.dt.bfloat16
F32 = mybir.dt.float32
F32R = mybir.dt.float32r
NP_BF16 = ml_dtypes.bfloat16

AV_STAGGER = 3       # S^T matmuls emitted ahead of the matching AV matmul


def _emit(tc: tile.TileContext, aps: dict):
    nc = tc.nc
    qT, kT, vT = aps["qT"], aps["kT"], aps["vT"]
    wq, wk, wv = aps["wq"], aps["wk"], aps["wv"]
    bias_pack, ident = aps["bias_pack"], aps["ident"]
    outT = aps["outT"]

    with tc.tile_pool(name="persist", bufs=1) as persist:
        # --- constants ---
        w_sb = {}
        for name, ap in (("wq", wq), ("wk", wk), ("wv", wv)):
            t = persist.tile([128, NMC, 128], BF16, tag=f"w_{name}", name=f"w_{name}")
            nc.sync.dma_start(t[:], ap.rearrange("(c p) d -> p c d", p=128))
            w_sb[name] = t
        bias_sb = persist.tile([128, 4], F32, tag="bias")
        nc.sync.dma_start(bias_sb[:], bias_pack[:])
        bq_ap = bias_sb[:, 0:1]
        bk_ap = bias_sb[:, 1:2]
        bv_ap = bias_sb[:, 2:3]
        ones_ap = bias_sb[:, 3:4]
        ident_sb = persist.tile([128, 128], BF16, tag="ident")
        nc.sync.dma_start(ident_sb[:], ident[:])

        # --- persistent activations ---
        kpT_t = [persist.tile([128, 128], BF16, tag=f"kpT{i}", name=f"kpT{i}")
                 for i in range(NKC)]
        qpT_t = [persist.tile([128, 512], BF16, tag=f"qpT{i}", name=f"qpT{i}")
                 for i in range(NQB)]
        vp_t = [persist.tile([128, 128], F32R, tag=f"vp{i}", name=f"vp{i}")
                for i in range(NKC)]

        # ---------- projections ----------
        with (
            tc.tile_pool(name="proj_ps", bufs=8, space="PSUM") as pps,
            tc.tile_pool(name="istream", bufs=2) as istream,
        ):
            def project(src_ap, w_name, n_cols):
                """Compute (w.T @ src) -> list of PSUM tiles [128, 512]."""
                nblk = n_cols // 512
                ps = [pps.tile([128, 512], F32, tag="pp", name=f"pp_{w_name}{i}")
                      for i in range(nblk)]
                for c in range(NMC):
                    xc = istream.tile([128, n_cols], BF16, tag="xs", name=f"xs_{w_name}{c}")
                    nc.sync.dma_start(xc[:], src_ap[c * 128:(c + 1) * 128, :])
                    for kb in range(nblk):
                        nc.tensor.matmul(
                            ps[kb][:],
                            lhsT=w_sb[w_name][:, c, :],
                            rhs=xc[:, kb * 512:(kb + 1) * 512],
                            start=(c == 0),
                            stop=(c == NMC - 1),
                        )
                return ps

            # kp^T [dk, S] -> chunk tiles (bf16, bias bk)
            ps = project(kT, "wk", S)
            for kb in range(NKB):
                for j in range(4):
                    nc.scalar.activation(
                        kpT_t[kb * 4 + j][:],
                        ps[kb][:, j * 128:(j + 1) * 128],
                        mybir.ActivationFunctionType.Identity,
                        bias=bk_ap,
                    )

            # vp^T [dv, S] (bias bv) then PE-transpose to vp [S, dv] chunks
            ps = project(vT, "wv", S)
            vpT_sb = []
            for kb in range(NKB):
                t = istream.tile([128, 512], BF16, tag="vpT", name=f"vpT{kb}")
                nc.scalar.activation(
                    t[:], ps[kb][:],
                    mybir.ActivationFunctionType.Identity, bias=bv_ap,
                )
                vpT_sb.append(t)
            for kc in range(NKC):
                tp = pps.tile([128, 128], BF16, tag="pp", name=f"tp{kc}")
                src = vpT_sb[kc // 4]
                nc.tensor.transpose(
                    tp[:], src[:, (kc % 4) * 128:(kc % 4 + 1) * 128], ident_sb[:]
                )
                nc.scalar.activation(
                    vp_t[kc][:], tp[:], mybir.ActivationFunctionType.Copy
                )

            # qp^T [dk, SQ] (bias/scale folded on host into wq,bq)
            ps = project(qT, "wq", SQ)
            for qb in range(NQB):
                nc.scalar.activation(
                    qpT_t[qb][:], ps[qb][:],
                    mybir.ActivationFunctionType.Identity, bias=bq_ap,
                )

        # ---------- attention ----------
        with (
            tc.tile_pool(name="s_ps", bufs=4, space="PSUM") as sps,
            tc.tile_pool(name="o_ps", bufs=4, space="PSUM") as ops,
            tc.tile_pool(name="e_sb", bufs=6) as epool,
            tc.tile_pool(name="acc_sb", bufs=4) as accpool,
            tc.tile_pool(name="misc_sb", bufs=4) as misc,
        ):
            o_tiles, acc_tiles = [], []
            for qb in range(NQB):
                ps_o = ops.tile([128, 512], F32, tag="po", name=f"po{qb}")
                sumacc = accpool.tile([128, 512], F32, tag="acc", name=f"acc{qb}")
                o_tiles.append(ps_o)
                acc_tiles.append(sumacc)

                pending = []  # (kc, ps_s) waiting for exp+AV

                def drain_one():
                    kc, ps_s = pending.pop(0)
                    e = epool.tile([128, 512], F32R, tag="e", name=f"e{qb}_{kc}")
                    nc.scalar.activation(
                        e[:], ps_s[:], mybir.ActivationFunctionType.Exp
                    )
                    if kc == 0:
                        nc.vector.tensor_copy(sumacc[:], e[:])
                    else:
                        nc.vector.tensor_add(sumacc[:], sumacc[:], e[:])
                    nc.tensor.matmul(
                        ps_o[:],
                        lhsT=vp_t[kc][:],
                        rhs=e[:],
                        start=(kc == 0),
                        stop=(kc == NKC - 1),
                    )

                for kc in range(NKC):
                    ps_s = sps.tile([128, 512], F32, tag="ps", name=f"ps{qb}_{kc}")
                    nc.tensor.matmul(
                        ps_s[:],
                        lhsT=kpT_t[kc][:],
                        rhs=qpT_t[qb][:],
                        start=True,
                        stop=True,
                    )
                    pending.append((kc, ps_s))
                    if len(pending) > AV_STAGGER:
                        drain_one()
                while pending:
                    drain_one()

            # deferred normalization tails
            for qb in range(NQB):
                ps_sum = sps.tile([1, 512], F32, tag="ps", name=f"psum{qb}")
                nc.tensor.matmul(
                    ps_sum[:],
                    lhsT=ones_ap,
                    rhs=acc_tiles[qb][:],
                    start=True,
                    stop=True,
                )
                recip = misc.tile([1, 512], F32, tag="recip", name=f"recip{qb}")
                nc.vector.reciprocal(recip[:], ps_sum[:])
                bcast = misc.tile([128, 512], F32, tag="bcast", name=f"bcast{qb}")
                nc.gpsimd.partition_broadcast(bcast[:], recip[:])
                out_sb = misc.tile([128, 512], F32, tag="out", name=f"out{qb}")
                nc.vector.tensor_mul(out_sb[:], o_tiles[qb][:], bcast[:])
                nc.sync.dma_start(outT[:, qb * 512:(qb + 1) * 512], out_sb[:])


_CACHE = {}


def _build():
    if "nc" in _CACHE:
        return _CACHE["nc"]
    nc = bacc.Bacc("TRN2", debug=False, num_devices=N_CORES)
    aps = {
        "qT": nc.dram_tensor("qT", [DM, SQ], BF16, kind="ExternalInput").ap(),
        "kT": nc.dram_tensor("kT", [DM, S], BF16, kind="ExternalInput").ap(),
        "vT": nc.dram_tensor("vT", [DM, S], BF16, kind="ExternalInput").ap(),
        "wq": nc.dram_tensor("wq", [DM, DK], BF16, kind="ExternalInput").ap(),
        "wk": nc.dram_tensor("wk", [DM, DK], BF16, kind="ExternalInput").ap(),
        "wv": nc.dram_tensor("wv", [DM, DV], BF16, kind="ExternalInput").ap(),
        "bias_pack": nc.dram_tensor(
            "bias_pack", [128, 4], F32, kind="ExternalInput"
        ).ap(),
        "ident": nc.dram_tensor("ident", [128, 128], BF16, kind="ExternalInput").ap(),
        "outT": nc.dram_tensor("outT", [DV, SQ], F32, kind="ExternalOutput").ap(),
    }
    with tile.TileContext(nc) as tc:
        _emit(tc, aps)
    nc.compile()
    _CACHE["nc"] = nc
    return nc


def make_in_maps(q, k, v, wq, bq, wk, bk, wv, bv):
    scale = 1.0 / math.sqrt(DK)
    wq_s = (np.asarray(wq, np.float32) * scale).astype(NP_BF16)
    wk_b = np.asarray(wk, np.float32).astype(NP_BF16)
    wv_b = np.asarray(wv, np.float32).astype(NP_BF16)
    bias_pack = np.zeros((128, 4), np.float32)
    bias_pack[:, 0] = np.asarray(bq, np.float32) * scale
    bias_pack[:, 1] = np.asarray(bk, np.float32)
    bias_pack[:, 2] = np.asarray(bv, np.float32)
    bias_pack[:, 3] = 1.0
    ident = np.eye(128, dtype=NP_BF16)

    in_maps = []
    for core in range(N_CORES):
        b, h = core // 2, core % 2
        qTb = np.ascontiguousarray(
            np.asarray(q[b], np.float32).T[:, h * SQ:(h + 1) * SQ]
        ).astype(NP_BF16)
        kTb = np.ascontiguousarray(np.asarray(k[b], np.float32).T).astype(NP_BF16)
        vTb = np.ascontiguousarray(np.asarray(v[b], np.float32).T).astype(NP_BF16)
        in_maps.append({
            "qT": qTb, "kT": kTb, "vT": vTb,
            "wq": wq_s, "wk": wk_b, "wv": wv_b,
            "bias_pack": bias_pack, "ident": ident,
        })
    return in_maps


def kernel(q, k, v, wq, bq, wk, bk, wv, bv, _trace=False, _tmpdir=None):
    nc = _build()
    in_maps = make_in_maps(q, k, v, wq, bq, wk, bk, wv, bv)
    res = run_bass_kernel_spmd(
        nc, in_maps, list(range(N_CORES)), trace=_trace, tmpdir=_tmpdir
    )
    out = np.empty((B, S, DV), np.float32)
    for core in range(N_CORES):
        b, h = core // 2, core % 2
        out[b, h * SQ:(h + 1) * SQ, :] = res.results[core]["outT"].T
    if _trace:
        kernel.last_results = res
    return out


# revision 6
# speedup vs baseline: 1.4259x; 1.4259x over previous
"""Bass/Trainium2 kernel for batched dot-product attention.

Problem: q,k,v [B=4, S=4096, D=1024]; projections to dk=dv=128; softmax
attention per batch element.  Sharded over 8 NeuronCores as (batch,
query-half): core c handles batch c//2, queries (c%2)*2048 ... +2048.

All layouts on-chip keep the contraction dimension on SBUF partitions:
  qT/kT/vT   [d_model, seq]   (host pre-transposed, bf16)
  kpT/qpT    [dk, seq]        (projection output, bf16)
  vp         [seq, dv]        (natural layout via PE transpose, bf16)
  S^T tiles  [keys, q]        (scores transposed, PSUM)
  out^T      [dv, q]          (final output transposed; host undoes)

Query blocks are processed in PAIRS (1024-wide exp tiles amortize the
ScalarE per-op overhead and halve AV weight loads).  Pair 0's attention
chunks are interleaved into the projection kb-loop so the TensorE stays
busy while kT/vT stream in.  Softmax denominators (sum over keys =
partition axis) via a ones-vector matmul; normalization via
partition_broadcast + reciprocal + multiply off the critical path.
Scale 1/sqrt(dk) is folded into wq/bq on the host.
"""

import math

import numpy as np
import ml_dtypes

import concourse.bass as bass
import concourse.tile as tile
from concourse import bacc, mybir
from concourse.bass_utils import run_bass_kernel_spmd

B, S, DM, DK, DV = 4, 4096, 1024, 128, 128
N_CORES = 8
SQ = S // 2          # queries per core
NQB = SQ // 512      # query blocks of 512 per core (4)
NKC = S // 128       # key chunks of 128 (32)
NMC = DM // 128      # d_model chunks (8)
NKB = S // 512       # key blocks of 512 (8)

BF16 = mybir.dt.bfloat16
F32 = mybir.dt.float32
F32R = mybir.dt.float32r
NP_BF16 = ml_dtypes.bfloat16

E_DT = F32R          # dtype of exp tiles (AV moving operand)
AV_STAGGER = 2       # pair-chunks the exp/AV drain lags the S matmuls

Identity = mybir.ActivationFunctionType.Identity
Copy = mybir.ActivationFunctionType.Copy
Exp = mybir.ActivationFunctionType.Exp


def _emit(tc: tile.TileContext, aps: dict):
    nc = tc.nc
    qT, kT, vT = aps["qT"], aps["kT"], aps["vT"]
    outT = aps["outT"]
    kT3 = kT.rearrange("(c p) s -> p c s", p=128)
    vT3 = vT.rearrange("(c p) s -> p c s", p=128)

    with tc.tile_pool(name="persist", bufs=1) as persist:
        # --- constants ---
        w_sb = {}
        for name in ("wq", "wk", "wv"):
            t = persist.tile([128, NMC, 128], BF16, tag=f"w_{name}", name=f"w_{name}")
            nc.sync.dma_start(t[:], aps[name].rearrange("(c p) d -> p c d", p=128))
            w_sb[name] = t
        bias_sb = persist.tile([128, 4], F32, tag="bias")
        nc.sync.dma_start(bias_sb[:], aps["bias_pack"][:])
        bq_ap, bk_ap, bv_ap = bias_sb[:, 0:1], bias_sb[:, 1:2], bias_sb[:, 2:3]
        ones_ap = bias_sb[:, 3:4]
        ident_sb = persist.tile([128, 128], BF16, tag="ident")
        nc.sync.dma_start(ident_sb[:], aps["ident"][:])

        # --- persistent activations ---
        kpT_blk = [persist.tile([128, 512], BF16, tag=f"kpT{i}", name=f"kpT{i}")
                   for i in range(NKB)]
        qpT_t = [persist.tile([128, 512], BF16, tag=f"qpT{i}", name=f"qpT{i}")
                 for i in range(NQB)]
        vp_pair = [persist.tile([128, 256], F32R, tag=f"vpp{i}", name=f"vpp{i}")
                   for i in range(NKC // 2)]

        with (
            tc.tile_pool(name="pp", bufs=2, space="PSUM") as pp,
            tc.tile_pool(name="sp", bufs=2, space="PSUM") as sp,
            tc.tile_pool(name="op", bufs=2, space="PSUM") as op,
            tc.tile_pool(name="xs", bufs=2) as xs,
            tc.tile_pool(name="ep", bufs=4) as ep,
            tc.tile_pool(name="accp", bufs=2) as accp,
            tc.tile_pool(name="miscp", bufs=2) as miscp,
        ):
            # ---- qp projection (whole qT in one DMA) ----
            qx = xs.tile([128, NMC, SQ], BF16, tag="qx", name="qx")
            nc.sync.dma_start(qx[:], qT.rearrange("(c p) s -> p c s", p=128))
            for qb in range(NQB):
                psq = pp.tile([128, 512], F32, tag="pp", name=f"psq{qb}")
                for c in range(NMC):
                    nc.tensor.matmul(
                        psq[:], lhsT=w_sb["wq"][:, c, :],
                        rhs=qx[:, c, qb * 512:(qb + 1) * 512],
                        start=(c == 0), stop=(c == NMC - 1),
                    )
                nc.vector.tensor_scalar_add(qpT_t[qb][:], psq[:], bq_ap)

            # ---- attention pair machinery ----
            def pair_begin(pidx):
                qa, qb_ = 2 * pidx, 2 * pidx + 1
                return dict(
                    p=pidx, qs=(qa, qb_),
                    o=[op.tile([128, 512], F32, tag="op", name=f"o{q}")
                       for q in (qa, qb_)],
                    acc=accp.tile([128, 1024], F32, tag="acc", name=f"acc{pidx}"),
                    pend=[],
                )

            def pair_drain(st):
                kc, s = st["pend"].pop(0)
                e = ep.tile([128, 1024], E_DT, tag="e", name=f"e{st['p']}_{kc}")
                nc.scalar.activation(e[:], s[:], Exp)
                if kc == 0:
                    nc.vector.tensor_copy(st["acc"][:], e[:].bitcast(F32))
                else:
                    nc.vector.tensor_add(st["acc"][:], st["acc"][:], e[:].bitcast(F32))
                vps = vp_pair[kc // 2][:, (kc % 2) * 128:(kc % 2 + 1) * 128]
                for h in range(2):
                    nc.tensor.matmul(
                        st["o"][h][:], lhsT=vps, rhs=e[:, h * 512:(h + 1) * 512],
                        start=(kc == 0), stop=(kc == NKC - 1),
                    )

            def pair_chunk(st, kc):
                s = sp.tile([128, 1024], F32, tag="sp", name=f"s{st['p']}_{kc}")
                kslice = kpT_blk[kc // 4][:, (kc % 4) * 128:(kc % 4 + 1) * 128]
                for h in range(2):
                    nc.tensor.matmul(
                        s[:, h * 512:(h + 1) * 512], lhsT=kslice,
                        rhs=qpT_t[st["qs"][h]][:], start=True, stop=True,
                    )
                st["pend"].append((kc, s))
                if len(st["pend"]) > AV_STAGGER:
                    pair_drain(st)

            def pair_tail(st):
                while st["pend"]:
                    pair_drain(st)
                for h, q in enumerate(st["qs"]):
                    ps_sum = sp.tile([1, 512], F32, tag="sp", name=f"pssum{q}")
                    nc.tensor.matmul(
                        ps_sum[:], lhsT=ones_ap,
                        rhs=st["acc"][:, h * 512:(h + 1) * 512],
                        start=True, stop=True,
                    )
                    row = miscp.tile([1, 512], F32, tag="row", name=f"row{q}")
                    nc.scalar.activation(row[:], ps_sum[:], Copy)
                    bc = miscp.tile([128, 512], F32, tag="bc", name=f"bc{q}")
                    nc.gpsimd.partition_broadcast(bc[:], row[:])
                    rec = miscp.tile([128, 512], F32, tag="rec", name=f"rec{q}")
                    nc.vector.reciprocal(rec[:], bc[:])
                    outsb = miscp.tile([128, 512], F32, tag="out", name=f"out{q}")
                    nc.vector.tensor_mul(outsb[:], st["o"][h][:], rec[:])
                    nc.sync.dma_start(outT[:, q * 512:(q + 1) * 512], outsb[:])

            # ---- kb loop: kp + vp projection, pair-0 attention interleaved ----
            st0 = pair_begin(0)
            for kb in range(NKB):
                kx = xs.tile([128, NMC, 512], BF16, tag="kx", name=f"kx{kb}")
                nc.sync.dma_start(kx[:], kT3[:, :, kb * 512:(kb + 1) * 512])
                psk = pp.tile([128, 512], F32, tag="pp", name=f"psk{kb}")
                for c in range(NMC):
                    nc.tensor.matmul(
                        psk[:], lhsT=w_sb["wk"][:, c, :], rhs=kx[:, c, :],
                        start=(c == 0), stop=(c == NMC - 1),
                    )
                nc.vector.tensor_scalar_add(kpT_blk[kb][:], psk[:], bk_ap)

                vx = xs.tile([128, NMC, 512], BF16, tag="vx", name=f"vx{kb}")
                nc.sync.dma_start(vx[:], vT3[:, :, kb * 512:(kb + 1) * 512])
                psv = pp.tile([128, 512], F32, tag="pp", name=f"psv{kb}")
                for c in range(NMC):
                    nc.tensor.matmul(
                        psv[:], lhsT=w_sb["wv"][:, c, :], rhs=vx[:, c, :],
                        start=(c == 0), stop=(c == NMC - 1),
                    )
                vpt = xs.tile([128, 512], BF16, tag="vpt", name=f"vpt{kb}")
                nc.vector.tensor_scalar_add(vpt[:], psv[:], bv_ap)
                for j in range(2):
                    tp = pp.tile([128, 256], BF16, tag="pp", name=f"tp{kb}_{j}")
                    for i in range(2):
                        nc.tensor.transpose(
                            tp[:, i * 128:(i + 1) * 128],
                            vpt[:, (2 * j + i) * 128:(2 * j + i + 1) * 128],
                            ident_sb[:],
                        )
                    nc.scalar.activation(vp_pair[2 * kb + j][:], tp[:], Copy)

                for kc in range(4 * kb, 4 * kb + 4):
                    pair_chunk(st0, kc)
            pair_tail(st0)

            # ---- pair 1 (pure attention, everything resident) ----
            st1 = pair_begin(1)
            for kc in range(NKC):
                pair_chunk(st1, kc)
            pair_tail(st1)


_CACHE = {}


def _build():
    if "nc" in _CACHE:
        return _CACHE["nc"]
    nc = bacc.Bacc("TRN2", debug=False, num_devices=N_CORES)
    aps = {
        "qT": nc.dram_tensor("qT", [DM, SQ], BF16, kind="ExternalInput").ap(),
        "kT": nc.dram_tensor("kT", [DM, S], BF16, kind="ExternalInput").ap(),
        "vT": nc.dram_tensor("vT", [DM, S], BF16, kind="ExternalInput").ap(),
        "wq": nc.dram_tensor("wq", [DM, DK], BF16, kind="ExternalInput").ap(),
        "wk": nc.dram_tensor("wk", [DM, DK], BF16, kind="ExternalInput").ap(),
        "wv": nc.dram_tensor("wv", [DM, DV], BF16, kind="ExternalInput").ap(),
        "bias_pack": nc.dram_tensor(
            "bias_pack", [128, 4], F32, kind="ExternalInput"
        ).ap(),
        "ident": nc.dram_tensor("ident", [128, 128], BF16, kind="ExternalInput").ap(),
        "outT": nc.dram_tensor("outT", [DV, SQ], F32, kind="ExternalOutput").ap(),
    }
    with tile.TileContext(nc) as tc:
        _emit(tc, aps)
    nc.compile()
    _CACHE["nc"] = nc
    return nc


def make_in_maps(q, k, v, wq, bq, wk, bk, wv, bv):
    scale = 1.0 / math.sqrt(DK)
    wq_s = (np.asarray(wq, np.float32) * scale).astype(NP_BF16)
    wk_b = np.asarray(wk, np.float32).astype(NP_BF16)
    wv_b = np.asarray(wv, np.float32).astype(NP_BF16)
    bias_pack = np.zeros((128, 4), np.float32)
    bias_pack[:, 0] = np.asarray(bq, np.float32) * scale
    bias_pack[:, 1] = np.asarray(bk, np.float32)
    bias_pack[:, 2] = np.asarray(bv, np.float32)
    bias_pack[:, 3] = 1.0
    ident = np.eye(128, dtype=NP_BF16)

    in_maps = []
    for core in range(N_CORES):
        b, h = core // 2, core % 2
        qTb = np.ascontiguousarray(
            np.asarray(q[b], np.float32).T[:, h * SQ:(h + 1) * SQ]
        ).astype(NP_BF16)
        kTb = np.ascontiguousarray(np.asarray(k[b], np.float32).T).astype(NP_BF16)
        vTb = np.ascontiguousarray(np.asarray(v[b], np.float32).T).astype(NP_BF16)
        in_maps.append({
            "qT": qTb, "kT": kTb, "vT": vTb,
            "wq": wq_s, "wk": wk_b, "wv": wv_b,
            "bias_pack": bias_pack, "ident": ident,
        })
    return in_maps


def kernel(q, k, v, wq, bq, wk, bk, wv, bv, _trace=False, _tmpdir=None):
    nc = _build()
    in_maps = make_in_maps(q, k, v, wq, bq, wk, bk, wv, bv)
    res = run_bass_kernel_spmd(
        nc, in_maps, list(range(N_CORES)), trace=_trace, tmpdir=_tmpdir
    )
    out = np.empty((B, S, DV), np.float32)
    for core in range(N_CORES):
        b, h = core // 2, core % 2
        out[b, h * SQ:(h + 1) * SQ, :] = res.results[core]["outT"].T
    if _trace:
        kernel.last_results = res
    return out


# revision 7
# speedup vs baseline: 1.7508x; 1.2279x over previous
"""Bass/Trainium2 kernel for batched dot-product attention.

Problem: q,k,v [B=4, S=4096, D=1024]; projections to dk=dv=128; softmax
attention per batch element.  Sharded over 8 NeuronCores as (batch,
query-half): core c handles batch c//2, queries (c%2)*2048 ... +2048.

All layouts on-chip keep the contraction dimension on SBUF partitions:
  qT/kT/vT   [d_model, seq]   (host pre-transposed, bf16)
  kpT/qpT    [dk, seq]        (projection output, bf16)
  vp         [seq, dv]        (natural layout via PE transpose, bf16)
  S^T tiles  [keys, q]        (scores transposed, PSUM)
  out^T      [dv, q]          (final output transposed; host undoes)

Query blocks are processed in PAIRS (1024-wide exp tiles amortize the
ScalarE per-op overhead and halve AV weight loads).  Pair 0's attention
chunks are interleaved into the projection kb-loop so the TensorE stays
busy while kT/vT stream in.  Softmax denominators (sum over keys =
partition axis) via a ones-vector matmul; normalization via
partition_broadcast + reciprocal + multiply off the critical path.
Scale 1/sqrt(dk) is folded into wq/bq on the host.
"""

import math

import numpy as np
import ml_dtypes

import concourse.bass as bass
import concourse.tile as tile
from concourse import bacc, mybir
from concourse.bass_utils import run_bass_kernel_spmd

B, S, DM, DK, DV = 4, 4096, 1024, 128, 128
N_CORES = 8
SQ = S // 2          # queries per core
NQB = SQ // 512      # query blocks of 512 per core (4)
NKC = S // 128       # key chunks of 128 (32)
NMC = DM // 128      # d_model chunks (8)
NKB = S // 512       # key blocks of 512 (8)

BF16 = mybir.dt.bfloat16
F32 = mybir.dt.float32
F32R = mybir.dt.float32r
NP_BF16 = ml_dtypes.bfloat16

E_DT = BF16          # dtype of exp tiles (AV moving operand)
AV_STAGGER = 2       # pair-chunks the exp/AV drain lags the S matmuls

Identity = mybir.ActivationFunctionType.Identity
Copy = mybir.ActivationFunctionType.Copy
Exp = mybir.ActivationFunctionType.Exp


def _emit(tc: tile.TileContext, aps: dict):
    nc = tc.nc
    qT, kT, vT = aps["qT"], aps["kT"], aps["vT"]
    outT = aps["outT"]
    kT3 = kT.rearrange("(c p) s -> p c s", p=128)
    vT3 = vT.rearrange("(c p) s -> p c s", p=128)

    with tc.tile_pool(name="persist", bufs=1) as persist:
        # --- constants ---
        w_sb = {}
        for name in ("wq", "wk", "wv"):
            t = persist.tile([128, NMC, 128], BF16, tag=f"w_{name}", name=f"w_{name}")
            nc.sync.dma_start(t[:], aps[name].rearrange("(c p) d -> p c d", p=128))
            w_sb[name] = t
        bias_sb = persist.tile([128, 4], F32, tag="bias")
        nc.sync.dma_start(bias_sb[:], aps["bias_pack"][:])
        bq_ap, bk_ap, bv_ap = bias_sb[:, 0:1], bias_sb[:, 1:2], bias_sb[:, 2:3]
        ones_ap = bias_sb[:, 3:4]
        ident_sb = persist.tile([128, 128], BF16, tag="ident")
        nc.sync.dma_start(ident_sb[:], aps["ident"][:])

        # --- persistent activations ---
        kpT_blk = [persist.tile([128, 512], BF16, tag=f"kpT{i}", name=f"kpT{i}")
                   for i in range(NKB)]
        qpT_t = [persist.tile([128, 512], BF16, tag=f"qpT{i}", name=f"qpT{i}")
                 for i in range(NQB)]
        vp_pair = [persist.tile([128, 256], BF16, tag=f"vpp{i}", name=f"vpp{i}")
                   for i in range(NKC // 2)]
        sums_sb = persist.tile([1, SQ], F32, tag="sums", name="sums_sb")

        with (
            tc.tile_pool(name="pp", bufs=2, space="PSUM") as pp,
            tc.tile_pool(name="sp", bufs=2, space="PSUM") as sp,
            tc.tile_pool(name="op", bufs=2, space="PSUM") as op,
            tc.tile_pool(name="xs", bufs=2) as xs,
            tc.tile_pool(name="ep", bufs=4) as ep,
            tc.tile_pool(name="accp", bufs=2) as accp,
            tc.tile_pool(name="miscp", bufs=2) as miscp,
        ):
            # ---- qp projection (whole qT in one DMA) ----
            qx = xs.tile([128, NMC, SQ], BF16, tag="qx", name="qx")
            nc.sync.dma_start(qx[:], qT.rearrange("(c p) s -> p c s", p=128))
            for qb in range(NQB):
                psq = pp.tile([128, 512], F32, tag="pp", name=f"psq{qb}")
                for c in range(NMC):
                    nc.tensor.matmul(
                        psq[:], lhsT=w_sb["wq"][:, c, :],
                        rhs=qx[:, c, qb * 512:(qb + 1) * 512],
                        start=(c == 0), stop=(c == NMC - 1),
                    )
                nc.scalar.activation(qpT_t[qb][:], psq[:], Identity, bias=bq_ap)

            # ---- attention pair machinery ----
            def pair_begin(pidx):
                qa, qb_ = 2 * pidx, 2 * pidx + 1
                return dict(
                    p=pidx, qs=(qa, qb_),
                    o=[op.tile([128, 512], F32, tag="op", name=f"o{q}")
                       for q in (qa, qb_)],
                    acc=accp.tile([128, 1024], F32, tag="acc", name=f"acc{pidx}"),
                    pend=[],
                )

            def pair_drain(st):
                kc, s = st["pend"].pop(0)
                e = ep.tile([128, 1024], E_DT, tag="e", name=f"e{st['p']}_{kc}")
                nc.scalar.activation(e[:], s[:], Exp)
                if kc % 2 == 0:
                    st["elast"] = e
                else:
                    # one bf16 add level halves the f32 accumulate traffic
                    tmp = ep.tile([128, 1024], BF16, tag="tmp", name=f"t{st['p']}_{kc}")
                    nc.vector.tensor_add(tmp[:], st["elast"][:], e[:])
                    if kc == 1:
                        nc.vector.tensor_copy(st["acc"][:], tmp[:])
                    else:
                        nc.vector.tensor_add(st["acc"][:], st["acc"][:], tmp[:])
                vps = vp_pair[kc // 2][:, (kc % 2) * 128:(kc % 2 + 1) * 128]
                for h in range(2):
                    nc.tensor.matmul(
                        st["o"][h][:], lhsT=vps, rhs=e[:, h * 512:(h + 1) * 512],
                        start=(kc == 0), stop=(kc == NKC - 1),
                    )

            def pair_chunk(st, kc):
                s = sp.tile([128, 1024], F32, tag="sp", name=f"s{st['p']}_{kc}")
                kslice = kpT_blk[kc // 4][:, (kc % 4) * 128:(kc % 4 + 1) * 128]
                for h in range(2):
                    nc.tensor.matmul(
                        s[:, h * 512:(h + 1) * 512], lhsT=kslice,
                        rhs=qpT_t[st["qs"][h]][:], start=True, stop=True,
                    )
                st["pend"].append((kc, s))
                if len(st["pend"]) > AV_STAGGER:
                    pair_drain(st)

            def pair_tail(st):
                while st["pend"]:
                    pair_drain(st)
                for h, q in enumerate(st["qs"]):
                    ps_sum = sp.tile([1, 512], F32, tag="sp", name=f"pssum{q}")
                    nc.tensor.matmul(
                        ps_sum[:], lhsT=ones_ap,
                        rhs=st["acc"][:, h * 512:(h + 1) * 512],
                        start=True, stop=True,
                    )
                    nc.scalar.activation(
                        sums_sb[:, q * 512:(q + 1) * 512], ps_sum[:], Copy
                    )
                    outsb = miscp.tile([128, 512], F32, tag="out", name=f"out{q}")
                    nc.scalar.activation(outsb[:], st["o"][h][:], Copy)
                    nc.sync.dma_start(outT[:, q * 512:(q + 1) * 512], outsb[:])

            # ---- kb loop: kp + vp projection, pair-0 attention interleaved ----
            st0 = pair_begin(0)
            for kb in range(NKB):
                kx = xs.tile([128, NMC, 512], BF16, tag="kx", name=f"kx{kb}")
                nc.sync.dma_start(kx[:], kT3[:, :, kb * 512:(kb + 1) * 512])
                psk = pp.tile([128, 512], F32, tag="pp", name=f"psk{kb}")
                for c in range(NMC):
                    nc.tensor.matmul(
                        psk[:], lhsT=w_sb["wk"][:, c, :], rhs=kx[:, c, :],
                        start=(c == 0), stop=(c == NMC - 1),
                    )
                nc.scalar.activation(kpT_blk[kb][:], psk[:], Identity, bias=bk_ap)

                vx = xs.tile([128, NMC, 512], BF16, tag="vx", name=f"vx{kb}")
                nc.sync.dma_start(vx[:], vT3[:, :, kb * 512:(kb + 1) * 512])
                psv = pp.tile([128, 512], F32, tag="pp", name=f"psv{kb}")
                for c in range(NMC):
                    nc.tensor.matmul(
                        psv[:], lhsT=w_sb["wv"][:, c, :], rhs=vx[:, c, :],
                        start=(c == 0), stop=(c == NMC - 1),
                    )
                vpt = xs.tile([128, 512], BF16, tag="vpt", name=f"vpt{kb}")
                nc.scalar.activation(vpt[:], psv[:], Identity, bias=bv_ap)
                for j in range(2):
                    tp = pp.tile([128, 256], BF16, tag="pp", name=f"tp{kb}_{j}")
                    for i in range(2):
                        nc.tensor.transpose(
                            tp[:, i * 128:(i + 1) * 128],
                            vpt[:, (2 * j + i) * 128:(2 * j + i + 1) * 128],
                            ident_sb[:],
                        )
                    nc.scalar.activation(vp_pair[2 * kb + j][:], tp[:], Copy)

                for kc in range(4 * kb, 4 * kb + 4):
                    pair_chunk(st0, kc)
            pair_tail(st0)

            # ---- pair 1 (pure attention, everything resident) ----
            st1 = pair_begin(1)
            for kc in range(NKC):
                pair_chunk(st1, kc)
            pair_tail(st1)
            nc.sync.dma_start(aps["sums"][:], sums_sb[:])


_CACHE = {}


def _build():
    if "nc" in _CACHE:
        return _CACHE["nc"]
    nc = bacc.Bacc("TRN2", debug=False, num_devices=N_CORES)
    aps = {
        "qT": nc.dram_tensor("qT", [DM, SQ], BF16, kind="ExternalInput").ap(),
        "kT": nc.dram_tensor("kT", [DM, S], BF16, kind="ExternalInput").ap(),
        "vT": nc.dram_tensor("vT", [DM, S], BF16, kind="ExternalInput").ap(),
        "wq": nc.dram_tensor("wq", [DM, DK], BF16, kind="ExternalInput").ap(),
        "wk": nc.dram_tensor("wk", [DM, DK], BF16, kind="ExternalInput").ap(),
        "wv": nc.dram_tensor("wv", [DM, DV], BF16, kind="ExternalInput").ap(),
        "bias_pack": nc.dram_tensor(
            "bias_pack", [128, 4], F32, kind="ExternalInput"
        ).ap(),
        "ident": nc.dram_tensor("ident", [128, 128], BF16, kind="ExternalInput").ap(),
        "outT": nc.dram_tensor("outT", [DV, SQ], F32, kind="ExternalOutput").ap(),
        "sums": nc.dram_tensor("sums", [1, SQ], F32, kind="ExternalOutput").ap(),
    }
    with tile.TileContext(nc) as tc:
        _emit(tc, aps)
    nc.compile()
    _CACHE["nc"] = nc
    return nc


def make_in_maps(q, k, v, wq, bq, wk, bk, wv, bv):
    scale = 1.0 / math.sqrt(DK)
    wq_s = (np.asarray(wq, np.float32) * scale).astype(NP_BF16)
    wk_b = np.asarray(wk, np.float32).astype(NP_BF16)
    wv_b = np.asarray(wv, np.float32).astype(NP_BF16)
    bias_pack = np.zeros((128, 4), np.float32)
    bias_pack[:, 0] = np.asarray(bq, np.float32) * scale
    bias_pack[:, 1] = np.asarray(bk, np.float32)
    bias_pack[:, 2] = np.asarray(bv, np.float32)
    bias_pack[:, 3] = 1.0
    ident = np.eye(128, dtype=NP_BF16)

    in_maps = []
    for core in range(N_CORES):
        b, h = core // 2, core % 2
        qTb = np.ascontiguousarray(
            np.asarray(q[b], np.float32).T[:, h * SQ:(h + 1) * SQ]
        ).astype(NP_BF16)
        kTb = np.ascontiguousarray(np.asarray(k[b], np.float32).T).astype(NP_BF16)
        vTb = np.ascontiguousarray(np.asarray(v[b], np.float32).T).astype(NP_BF16)
        in_maps.append({
            "qT": qTb, "kT": kTb, "vT": vTb,
            "wq": wq_s, "wk": wk_b, "wv": wv_b,
            "bias_pack": bias_pack, "ident": ident,
        })
    return in_maps


def kernel(q, k, v, wq, bq, wk, bk, wv, bv, _trace=False, _tmpdir=None):
    nc = _build()
    in_maps = make_in_maps(q, k, v, wq, bq, wk, bk, wv, bv)
    res = run_bass_kernel_spmd(
        nc, in_maps, list(range(N_CORES)), trace=_trace, tmpdir=_tmpdir
    )
    out = np.empty((B, S, DV), np.float32)
    for core in range(N_CORES):
        b, h = core // 2, core % 2
        r = res.results[core]
        out[b, h * SQ:(h + 1) * SQ, :] = (r["outT"] / r["sums"]).T
    if _trace:
        kernel.last_results = res
    return out


# revision 9
# speedup vs baseline: 1.7683x; 1.0100x over previous
"""Bass/Trainium2 kernel for batched dot-product attention.

Problem: q,k,v [B=4, S=4096, D=1024]; projections to dk=dv=128; softmax
attention per batch element.  Sharded over 8 NeuronCores as (batch,
query-half): core c handles batch c//2, queries (c%2)*2048 ... +2048.

All layouts on-chip keep the contraction dimension on SBUF partitions:
  qT/kT/vT   [d_model, seq]   (host pre-transposed, bf16)
  kpT/qpT    [dk, seq]        (projection output, bf16)
  vp         [seq, dv]        (natural layout via PE transpose, bf16)
  S^T tiles  [keys, q]        (scores transposed, PSUM)
  out^T      [dv, q]          (final output transposed; host undoes)

Query blocks are processed in PAIRS (1024-wide exp tiles amortize the
ScalarE per-op overhead and halve AV weight loads).  Pair 0's attention
chunks are interleaved into the projection kb-loop so the TensorE stays
busy while kT/vT stream in.  Softmax denominators (sum over keys =
partition axis) via a ones-vector matmul; normalization via
partition_broadcast + reciprocal + multiply off the critical path.
Scale 1/sqrt(dk) is folded into wq/bq on the host.
"""

import math

import numpy as np
import ml_dtypes

import concourse.bass as bass
import concourse.tile as tile
from concourse import bacc, mybir
from concourse.bass_utils import run_bass_kernel_spmd

B, S, DM, DK, DV = 4, 4096, 1024, 128, 128
N_CORES = 8
SQ = S // 2          # queries per core
NQB = SQ // 512      # query blocks of 512 per core (4)
NKC = S // 128       # key chunks of 128 (32)
NMC = DM // 128      # d_model chunks (8)
NKB = S // 512       # key blocks of 512 (8)

BF16 = mybir.dt.bfloat16
F32 = mybir.dt.float32
F32R = mybir.dt.float32r
NP_BF16 = ml_dtypes.bfloat16

E_DT = BF16          # dtype of exp tiles (AV moving operand)
AV_STAGGER = 2       # pair-chunks the exp/AV drain lags the S matmuls

Identity = mybir.ActivationFunctionType.Identity
Copy = mybir.ActivationFunctionType.Copy
Exp = mybir.ActivationFunctionType.Exp


def _emit(tc: tile.TileContext, aps: dict):
    nc = tc.nc
    qT, kT, vT = aps["qT"], aps["kT"], aps["vT"]
    outT = aps["outT"]
    kT3 = kT.rearrange("(c p) s -> p c s", p=128)
    vT3 = vT.rearrange("(c p) s -> p c s", p=128)

    with tc.tile_pool(name="persist", bufs=1) as persist:
        # --- constants ---
        w_sb = {}
        for name in ("wq", "wk", "wv"):
            t = persist.tile([128, NMC, 128], BF16, tag=f"w_{name}", name=f"w_{name}")
            nc.sync.dma_start(t[:], aps[name].rearrange("(c p) d -> p c d", p=128))
            w_sb[name] = t
        bias_sb = persist.tile([128, 4], F32, tag="bias")
        nc.sync.dma_start(bias_sb[:], aps["bias_pack"][:])
        bq_ap, bk_ap, bv_ap = bias_sb[:, 0:1], bias_sb[:, 1:2], bias_sb[:, 2:3]
        ones_ap = bias_sb[:, 3:4]
        ident_sb = persist.tile([128, 128], BF16, tag="ident")
        nc.sync.dma_start(ident_sb[:], aps["ident"][:])

        # --- persistent activations ---
        kpT_blk = [persist.tile([128, 512], BF16, tag=f"kpT{i}", name=f"kpT{i}")
                   for i in range(NKB)]
        qpT_t = [persist.tile([128, 512], BF16, tag=f"qpT{i}", name=f"qpT{i}")
                 for i in range(NQB)]
        vp_pair = [persist.tile([128, 256], BF16, tag=f"vpp{i}", name=f"vpp{i}")
                   for i in range(NKC // 2)]
        sums_sb = persist.tile([1, SQ], F32, tag="sums", name="sums_sb")

        with (
            tc.tile_pool(name="op", bufs=2, space="PSUM") as op,
            tc.tile_pool(name="ep", bufs=4) as ep,
            tc.tile_pool(name="accp", bufs=2) as accp,
            tc.tile_pool(name="miscp", bufs=2) as miscp,
        ):
            inner = tc.tile_pool(name="pp", bufs=2, space="PSUM")
            pp = inner.__enter__()
            _sp_cm = tc.tile_pool(name="sp", bufs=2, space="PSUM")
            sp = _sp_cm.__enter__()
            _xs_cm = tc.tile_pool(name="xs", bufs=2)
            xs = _xs_cm.__enter__()

            # ---- qp projection (block-sized DMAs for a fast start) ----
            qT3 = qT.rearrange("(c p) s -> p c s", p=128)
            for qb in range(NQB):
                qx = xs.tile([128, NMC, 512], BF16, tag="qx", name=f"qx{qb}")
                nc.sync.dma_start(qx[:], qT3[:, :, qb * 512:(qb + 1) * 512])
                psq = pp.tile([128, 512], F32, tag="pp", name=f"psq{qb}")
                for c in range(NMC):
                    nc.tensor.matmul(
                        psq[:], lhsT=w_sb["wq"][:, c, :],
                        rhs=qx[:, c, :],
                        start=(c == 0), stop=(c == NMC - 1),
                    )
                nc.vector.tensor_scalar_add(qpT_t[qb][:], psq[:], bq_ap)

            # ---- attention pair machinery ----
            def pair_begin(pidx, spool):
                qa, qb_ = 2 * pidx, 2 * pidx + 1
                return dict(
                    p=pidx, qs=(qa, qb_), sp=spool,
                    o=[op.tile([128, 512], F32, tag="op", name=f"o{q}")
                       for q in (qa, qb_)],
                    acc=accp.tile([128, 1024], F32, tag="acc", name=f"acc{pidx}"),
                    pend=[],
                )

            def pair_drain(st):
                kc, s = st["pend"].pop(0)
                e = ep.tile([128, 1024], E_DT, tag="e", name=f"e{st['p']}_{kc}")
                nc.scalar.activation(e[:], s[:], Exp)
                if kc % 2 == 0:
                    st["elast"] = e
                else:
                    # one bf16 add level halves the f32 accumulate traffic
                    tmp = ep.tile([128, 1024], BF16, tag="tmp", name=f"t{st['p']}_{kc}")
                    nc.vector.tensor_add(tmp[:], st["elast"][:], e[:])
                    if kc == 1:
                        nc.vector.tensor_copy(st["acc"][:], tmp[:])
                    else:
                        nc.vector.tensor_add(st["acc"][:], st["acc"][:], tmp[:])
                vps = vp_pair[kc // 2][:, (kc % 2) * 128:(kc % 2 + 1) * 128]
                for h in range(2):
                    nc.tensor.matmul(
                        st["o"][h][:], lhsT=vps, rhs=e[:, h * 512:(h + 1) * 512],
                        start=(kc == 0), stop=(kc == NKC - 1),
                    )

            def pair_chunk(st, kc):
                s = st["sp"].tile([128, 1024], F32, tag="sp", name=f"s{st['p']}_{kc}")
                kslice = kpT_blk[kc // 4][:, (kc % 4) * 128:(kc % 4 + 1) * 128]
                for h in range(2):
                    nc.tensor.matmul(
                        s[:, h * 512:(h + 1) * 512], lhsT=kslice,
                        rhs=qpT_t[st["qs"][h]][:], start=True, stop=True,
                    )
                st["pend"].append((kc, s))
                if len(st["pend"]) > AV_STAGGER:
                    pair_drain(st)

            def pair_tail(st):
                while st["pend"]:
                    pair_drain(st)
                for h, q in enumerate(st["qs"]):
                    ps_sum = st["sp"].tile([1, 512], F32, tag="sp", name=f"pssum{q}")
                    nc.tensor.matmul(
                        ps_sum[:], lhsT=ones_ap,
                        rhs=st["acc"][:, h * 512:(h + 1) * 512],
                        start=True, stop=True,
                    )
                    nc.scalar.activation(
                        sums_sb[:, q * 512:(q + 1) * 512], ps_sum[:], Copy
                    )
                    outsb = miscp.tile([128, 512], F32, tag="out", name=f"out{q}")
                    nc.scalar.activation(outsb[:], st["o"][h][:], Copy)
                    nc.sync.dma_start(outT[:, q * 512:(q + 1) * 512], outsb[:])

            # ---- kb loop: kp + vp projection, pair-0 attention interleaved ----
            st0 = pair_begin(0, sp)
            for kb in range(NKB):
                kx = xs.tile([128, NMC, 512], BF16, tag="kx", name=f"kx{kb}")
                nc.sync.dma_start(kx[:], kT3[:, :, kb * 512:(kb + 1) * 512])
                psk = pp.tile([128, 512], F32, tag="pp", name=f"psk{kb}")
                for c in range(NMC):
                    nc.tensor.matmul(
                        psk[:], lhsT=w_sb["wk"][:, c, :], rhs=kx[:, c, :],
                        start=(c == 0), stop=(c == NMC - 1),
                    )
                nc.vector.tensor_scalar_add(kpT_blk[kb][:], psk[:], bk_ap)

                vx = xs.tile([128, NMC, 512], BF16, tag="vx", name=f"vx{kb}")
                nc.sync.dma_start(vx[:], vT3[:, :, kb * 512:(kb + 1) * 512])
                psv = pp.tile([128, 512], F32, tag="pp", name=f"psv{kb}")
                for c in range(NMC):
                    nc.tensor.matmul(
                        psv[:], lhsT=w_sb["wv"][:, c, :], rhs=vx[:, c, :],
                        start=(c == 0), stop=(c == NMC - 1),
                    )
                vpt = xs.tile([128, 512], BF16, tag="vpt", name=f"vpt{kb}")
                nc.vector.tensor_scalar_add(vpt[:], psv[:], bv_ap)
                for j in range(2):
                    tp = pp.tile([128, 256], BF16, tag="pp", name=f"tp{kb}_{j}")
                    for i in range(2):
                        nc.tensor.transpose(
                            tp[:, i * 128:(i + 1) * 128],
                            vpt[:, (2 * j + i) * 128:(2 * j + i + 1) * 128],
                            ident_sb[:],
                        )
                    nc.vector.tensor_copy(vp_pair[2 * kb + j][:], tp[:])

                for kc in range(4 * kb, 4 * kb + 4):
                    pair_chunk(st0, kc)
            pair_tail(st0)
            _xs_cm.__exit__(None, None, None)
            _sp_cm.__exit__(None, None, None)
            inner.__exit__(None, None, None)

            # ---- pair 1 (pure attention, everything resident) ----
            with tc.tile_pool(name="sp2", bufs=3, space="PSUM") as sp2:
                st1 = pair_begin(1, sp2)
                for kc in range(NKC):
                    pair_chunk(st1, kc)
                pair_tail(st1)
            nc.sync.dma_start(aps["sums"][:], sums_sb[:])


_CACHE = {}


def _build():
    if "nc" in _CACHE:
        return _CACHE["nc"]
    nc = bacc.Bacc("TRN2", debug=False, num_devices=N_CORES)
    aps = {
        "qT": nc.dram_tensor("qT", [DM, SQ], BF16, kind="ExternalInput").ap(),
        "kT": nc.dram_tensor("kT", [DM, S], BF16, kind="ExternalInput").ap(),
        "vT": nc.dram_tensor("vT", [DM, S], BF16, kind="ExternalInput").ap(),
        "wq": nc.dram_tensor("wq", [DM, DK], BF16, kind="ExternalInput").ap(),
        "wk": nc.dram_tensor("wk", [DM, DK], BF16, kind="ExternalInput").ap(),
        "wv": nc.dram_tensor("wv", [DM, DV], BF16, kind="ExternalInput").ap(),
        "bias_pack": nc.dram_tensor(
            "bias_pack", [128, 4], F32, kind="ExternalInput"
        ).ap(),
        "ident": nc.dram_tensor("ident", [128, 128], BF16, kind="ExternalInput").ap(),
        "outT": nc.dram_tensor("outT", [DV, SQ], F32, kind="ExternalOutput").ap(),
        "sums": nc.dram_tensor("sums", [1, SQ], F32, kind="ExternalOutput").ap(),
    }
    with tile.TileContext(nc) as tc:
        _emit(tc, aps)
    nc.compile()
    _CACHE["nc"] = nc
    return nc


def make_in_maps(q, k, v, wq, bq, wk, bk, wv, bv):
    scale = 1.0 / math.sqrt(DK)
    wq_s = (np.asarray(wq, np.float32) * scale).astype(NP_BF16)
    wk_b = np.asarray(wk, np.float32).astype(NP_BF16)
    wv_b = np.asarray(wv, np.float32).astype(NP_BF16)
    bias_pack = np.zeros((128, 4), np.float32)
    bias_pack[:, 0] = np.asarray(bq, np.float32) * scale
    bias_pack[:, 1] = np.asarray(bk, np.float32)
    bias_pack[:, 2] = np.asarray(bv, np.float32)
    bias_pack[:, 3] = 1.0
    ident = np.eye(128, dtype=NP_BF16)

    in_maps = []
    for core in range(N_CORES):
        b, h = core // 2, core % 2
        qTb = np.ascontiguousarray(
            np.asarray(q[b], np.float32).T[:, h * SQ:(h + 1) * SQ]
        ).astype(NP_BF16)
        kTb = np.ascontiguousarray(np.asarray(k[b], np.float32).T).astype(NP_BF16)
        vTb = np.ascontiguousarray(np.asarray(v[b], np.float32).T).astype(NP_BF16)
        in_maps.append({
            "qT": qTb, "kT": kTb, "vT": vTb,
            "wq": wq_s, "wk": wk_b, "wv": wv_b,
            "bias_pack": bias_pack, "ident": ident,
        })
    return in_maps


def kernel(q, k, v, wq, bq, wk, bk, wv, bv, _trace=False, _tmpdir=None):
    nc = _build()
    in_maps = make_in_maps(q, k, v, wq, bq, wk, bk, wv, bv)
    res = run_bass_kernel_spmd(
        nc, in_maps, list(range(N_CORES)), trace=_trace, tmpdir=_tmpdir
    )
    out = np.empty((B, S, DV), np.float32)
    for core in range(N_CORES):
        b, h = core // 2, core % 2
        r = res.results[core]
        out[b, h * SQ:(h + 1) * SQ, :] = (r["outT"] / r["sums"]).T
    if _trace:
        kernel.last_results = res
    return out


# revision 10
# speedup vs baseline: 1.7940x; 1.0145x over previous
"""Bass/Trainium2 kernel for batched dot-product attention.

Problem: q,k,v [B=4, S=4096, D=1024]; projections to dk=dv=128; softmax
attention per batch element.  Sharded over 8 NeuronCores as (batch,
query-half): core c handles batch c//2, queries (c%2)*2048 ... +2048.

All layouts on-chip keep the contraction dimension on SBUF partitions:
  qT/kT/vT   [d_model, seq]   (host pre-transposed, bf16)
  kpT/qpT    [dk, seq]        (projection output, bf16)
  vp         [seq, dv]        (natural layout via PE transpose, bf16)
  S^T tiles  [keys, q]        (scores transposed, PSUM)
  out^T      [dv, q]          (final output transposed; host undoes)

Query blocks are processed in PAIRS (1024-wide exp tiles amortize the
ScalarE per-op overhead and halve AV weight loads).  Pair 0's attention
chunks are interleaved into the projection kb-loop so the TensorE stays
busy while kT/vT stream in.  Softmax denominators (sum over keys =
partition axis) via a ones-vector matmul; normalization via
partition_broadcast + reciprocal + multiply off the critical path.
Scale 1/sqrt(dk) is folded into wq/bq on the host.
"""

import math

import numpy as np
import ml_dtypes

import concourse.bass as bass
import concourse.tile as tile
from concourse import bacc, mybir
from concourse.bass_utils import run_bass_kernel_spmd

B, S, DM, DK, DV = 4, 4096, 1024, 128, 128
N_CORES = 8
SQ = S // 2          # queries per core
NQB = SQ // 512      # query blocks of 512 per core (4)
NKC = S // 128       # key chunks of 128 (32)
NMC = DM // 128      # d_model chunks (8)
NKB = S // 512       # key blocks of 512 (8)

BF16 = mybir.dt.bfloat16
F32 = mybir.dt.float32
F32R = mybir.dt.float32r
NP_BF16 = ml_dtypes.bfloat16

E_DT = BF16          # dtype of exp tiles (AV moving operand)
AV_STAGGER = 2       # pair-chunks the exp/AV drain lags the S matmuls

Identity = mybir.ActivationFunctionType.Identity
Copy = mybir.ActivationFunctionType.Copy
Exp = mybir.ActivationFunctionType.Exp


def _emit(tc: tile.TileContext, aps: dict):
    nc = tc.nc
    qT, kT, vT = aps["qT"], aps["kT"], aps["vT"]
    outT = aps["outT"]

    with tc.tile_pool(name="persist", bufs=1) as persist:
        # --- constants ---
        w_sb = {}
        for name in ("wq", "wk", "wv"):
            t = persist.tile([128, NMC, 128], BF16, tag=f"w_{name}", name=f"w_{name}")
            nc.scalar.dma_start(t[:], aps[name][:])
            w_sb[name] = t
        bias_sb = persist.tile([128, 4], F32, tag="bias")
        nc.scalar.dma_start(bias_sb[:], aps["bias_pack"][:])
        bq_ap, bk_ap, bv_ap = bias_sb[:, 0:1], bias_sb[:, 1:2], bias_sb[:, 2:3]
        ones_ap = bias_sb[:, 3:4]
        ident_sb = persist.tile([128, 128], BF16, tag="ident")
        nc.scalar.dma_start(ident_sb[:], aps["ident"][:])

        # --- persistent activations ---
        kpT_blk = [persist.tile([128, 512], BF16, tag=f"kpT{i}", name=f"kpT{i}")
                   for i in range(NKB)]
        qpT_t = [persist.tile([128, 512], BF16, tag=f"qpT{i}", name=f"qpT{i}")
                 for i in range(NQB)]
        vp_pair = [persist.tile([128, 256], BF16, tag=f"vpp{i}", name=f"vpp{i}")
                   for i in range(NKC // 2)]
        sums_sb = persist.tile([1, SQ], F32, tag="sums", name="sums_sb")

        with (
            tc.tile_pool(name="op", bufs=2, space="PSUM") as op,
            tc.tile_pool(name="ep", bufs=4) as ep,
            tc.tile_pool(name="accp", bufs=2) as accp,
            tc.tile_pool(name="miscp", bufs=2) as miscp,
        ):
            inner = tc.tile_pool(name="pp", bufs=2, space="PSUM")
            pp = inner.__enter__()
            _sp_cm = tc.tile_pool(name="sp", bufs=2, space="PSUM")
            sp = _sp_cm.__enter__()
            _xs_cm = tc.tile_pool(name="xs", bufs=2)
            xs = _xs_cm.__enter__()

            # ---- qp projection (block-sized DMAs for a fast start) ----
            qxs = []
            for qb in range(NQB):
                qx = xs.tile([128, NMC, 512], BF16, tag="qx", name=f"qx{qb}",
                             bufs=4)
                nc.sync.dma_start(qx[:], qT[qb])
                qxs.append(qx)
            kxs, vxs = {}, {}

            def fetch_stripe(kb):
                kx = xs.tile([128, NMC, 512], BF16, tag="kx", name=f"kx{kb}",
                             bufs=3)
                nc.sync.dma_start(kx[:], kT[kb])
                vx = xs.tile([128, NMC, 512], BF16, tag="vx", name=f"vx{kb}",
                             bufs=3)
                nc.sync.dma_start(vx[:], vT[kb])
                kxs[kb], vxs[kb] = kx, vx

            fetch_stripe(0)
            for qb in range(NQB):
                qx = qxs[qb]
                psq = pp.tile([128, 512], F32, tag="pp", name=f"psq{qb}")
                for c in range(NMC):
                    nc.tensor.matmul(
                        psq[:], lhsT=w_sb["wq"][:, c, :],
                        rhs=qx[:, c, :],
                        start=(c == 0), stop=(c == NMC - 1),
                    )
                nc.vector.tensor_scalar_add(qpT_t[qb][:], psq[:], bq_ap)

            # ---- attention pair machinery ----
            def pair_begin(pidx, spool):
                qa, qb_ = 2 * pidx, 2 * pidx + 1
                return dict(
                    p=pidx, qs=(qa, qb_), sp=spool,
                    o=[op.tile([128, 512], F32, tag="op", name=f"o{q}")
                       for q in (qa, qb_)],
                    acc=accp.tile([128, 1024], F32, tag="acc", name=f"acc{pidx}"),
                    pend=[],
                )

            def pair_drain(st):
                kc, s = st["pend"].pop(0)
                e = ep.tile([128, 1024], E_DT, tag="e", name=f"e{st['p']}_{kc}")
                nc.scalar.activation(e[:], s[:], Exp)
                if kc % 2 == 0:
                    st["elast"] = e
                else:
                    # one bf16 add level halves the f32 accumulate traffic
                    tmp = ep.tile([128, 1024], BF16, tag="tmp", name=f"t{st['p']}_{kc}")
                    nc.vector.tensor_add(tmp[:], st["elast"][:], e[:])
                    if kc == 1:
                        nc.vector.tensor_copy(st["acc"][:], tmp[:])
                    else:
                        nc.vector.tensor_add(st["acc"][:], st["acc"][:], tmp[:])
                vps = vp_pair[kc // 2][:, (kc % 2) * 128:(kc % 2 + 1) * 128]
                for h in range(2):
                    nc.tensor.matmul(
                        st["o"][h][:], lhsT=vps, rhs=e[:, h * 512:(h + 1) * 512],
                        start=(kc == 0), stop=(kc == NKC - 1),
                    )

            def pair_chunk(st, kc):
                s = st["sp"].tile([128, 1024], F32, tag="sp", name=f"s{st['p']}_{kc}")
                kslice = kpT_blk[kc // 4][:, (kc % 4) * 128:(kc % 4 + 1) * 128]
                for h in range(2):
                    nc.tensor.matmul(
                        s[:, h * 512:(h + 1) * 512], lhsT=kslice,
                        rhs=qpT_t[st["qs"][h]][:], start=True, stop=True,
                    )
                st["pend"].append((kc, s))
                if len(st["pend"]) > AV_STAGGER:
                    pair_drain(st)

            def pair_tail(st):
                while st["pend"]:
                    pair_drain(st)
                for h, q in enumerate(st["qs"]):
                    ps_sum = st["sp"].tile([1, 512], F32, tag="sp", name=f"pssum{q}")
                    nc.tensor.matmul(
                        ps_sum[:], lhsT=ones_ap,
                        rhs=st["acc"][:, h * 512:(h + 1) * 512],
                        start=True, stop=True,
                    )
                    nc.scalar.activation(
                        sums_sb[:, q * 512:(q + 1) * 512], ps_sum[:], Copy
                    )
                    outsb = miscp.tile([128, 512], F32, tag="out", name=f"out{q}")
                    nc.scalar.activation(outsb[:], st["o"][h][:], Copy)
                    nc.scalar.dma_start(outT[:, q * 512:(q + 1) * 512], outsb[:])

            # ---- kb loop: kp + vp projection, pair-0 attention interleaved ----
            st0 = pair_begin(0, sp)
            for kb in range(NKB):
                if kb + 1 < NKB:
                    fetch_stripe(kb + 1)
                kx = kxs.pop(kb)
                psk = pp.tile([128, 512], F32, tag="pp", name=f"psk{kb}")
                for c in range(NMC):
                    nc.tensor.matmul(
                        psk[:], lhsT=w_sb["wk"][:, c, :], rhs=kx[:, c, :],
                        start=(c == 0), stop=(c == NMC - 1),
                    )
                nc.vector.tensor_scalar_add(kpT_blk[kb][:], psk[:], bk_ap)

                vx = vxs.pop(kb)
                psv = pp.tile([128, 512], F32, tag="pp", name=f"psv{kb}")
                for c in range(NMC):
                    nc.tensor.matmul(
                        psv[:], lhsT=w_sb["wv"][:, c, :], rhs=vx[:, c, :],
                        start=(c == 0), stop=(c == NMC - 1),
                    )
                vpt = xs.tile([128, 512], BF16, tag="vpt", name=f"vpt{kb}")
                nc.vector.tensor_scalar_add(vpt[:], psv[:], bv_ap)
                for j in range(2):
                    tp = pp.tile([128, 256], BF16, tag="pp", name=f"tp{kb}_{j}")
                    for i in range(2):
                        nc.tensor.transpose(
                            tp[:, i * 128:(i + 1) * 128],
                            vpt[:, (2 * j + i) * 128:(2 * j + i + 1) * 128],
                            ident_sb[:],
                        )
                    nc.vector.tensor_copy(vp_pair[2 * kb + j][:], tp[:])

                for kc in range(4 * kb, 4 * kb + 4):
                    pair_chunk(st0, kc)
            pair_tail(st0)
            _xs_cm.__exit__(None, None, None)
            _sp_cm.__exit__(None, None, None)
            inner.__exit__(None, None, None)

            # ---- pair 1 (pure attention, everything resident) ----
            with tc.tile_pool(name="sp2", bufs=3, space="PSUM") as sp2:
                st1 = pair_begin(1, sp2)
                for kc in range(NKC):
                    pair_chunk(st1, kc)
                pair_tail(st1)
            nc.scalar.dma_start(aps["sums"][:], sums_sb[:])


_CACHE = {}


def _build():
    if "nc" in _CACHE:
        return _CACHE["nc"]
    nc = bacc.Bacc("TRN2", debug=False, num_devices=N_CORES)
    aps = {
        "qT": nc.dram_tensor("qT", [NQB, 128, NMC, 512], BF16,
                             kind="ExternalInput").ap(),
        "kT": nc.dram_tensor("kT", [NKB, 128, NMC, 512], BF16,
                             kind="ExternalInput").ap(),
        "vT": nc.dram_tensor("vT", [NKB, 128, NMC, 512], BF16,
                             kind="ExternalInput").ap(),
        "wq": nc.dram_tensor("wq", [128, NMC, DK], BF16, kind="ExternalInput").ap(),
        "wk": nc.dram_tensor("wk", [128, NMC, DK], BF16, kind="ExternalInput").ap(),
        "wv": nc.dram_tensor("wv", [128, NMC, DV], BF16, kind="ExternalInput").ap(),
        "bias_pack": nc.dram_tensor(
            "bias_pack", [128, 4], F32, kind="ExternalInput"
        ).ap(),
        "ident": nc.dram_tensor("ident", [128, 128], BF16, kind="ExternalInput").ap(),
        "outT": nc.dram_tensor("outT", [DV, SQ], F32, kind="ExternalOutput").ap(),
        "sums": nc.dram_tensor("sums", [1, SQ], F32, kind="ExternalOutput").ap(),
    }
    with tile.TileContext(nc) as tc:
        _emit(tc, aps)
    nc.compile()
    _CACHE["nc"] = nc
    return nc


def _pack_w(w):
    # [DM, d] -> [128, NMC, d]  (chunk-major weight layout)
    return np.ascontiguousarray(np.asarray(w).reshape(NMC, 128, -1).transpose(1, 0, 2))


def _pack_x(xT, nblk):
    # [DM, n] -> [nblk, 128, NMC, 512]  (contiguous per-stripe layout)
    return np.ascontiguousarray(
        xT.reshape(NMC, 128, nblk, 512).transpose(2, 1, 0, 3))


def make_in_maps(q, k, v, wq, bq, wk, bk, wv, bv):
    scale = 1.0 / math.sqrt(DK)
    wq_s = _pack_w((np.asarray(wq, np.float32) * scale).astype(NP_BF16))
    wk_b = _pack_w(np.asarray(wk, np.float32).astype(NP_BF16))
    wv_b = _pack_w(np.asarray(wv, np.float32).astype(NP_BF16))
    bias_pack = np.zeros((128, 4), np.float32)
    bias_pack[:, 0] = np.asarray(bq, np.float32) * scale
    bias_pack[:, 1] = np.asarray(bk, np.float32)
    bias_pack[:, 2] = np.asarray(bv, np.float32)
    bias_pack[:, 3] = 1.0
    ident = np.eye(128, dtype=NP_BF16)

    in_maps = []
    for core in range(N_CORES):
        b, h = core // 2, core % 2
        qTb = _pack_x(
            np.asarray(q[b], np.float32).T[:, h * SQ:(h + 1) * SQ].astype(NP_BF16),
            NQB)
        kTb = _pack_x(np.asarray(k[b], np.float32).T.astype(NP_BF16), NKB)
        vTb = _pack_x(np.asarray(v[b], np.float32).T.astype(NP_BF16), NKB)
        in_maps.append({
            "qT": qTb, "kT": kTb, "vT": vTb,
            "wq": wq_s, "wk": wk_b, "wv": wv_b,
            "bias_pack": bias_pack, "ident": ident,
        })
    return in_maps


def kernel(q, k, v, wq, bq, wk, bk, wv, bv, _trace=False, _tmpdir=None):
    nc = _build()
    in_maps = make_in_maps(q, k, v, wq, bq, wk, bk, wv, bv)
    res = run_bass_kernel_spmd(
        nc, in_maps, list(range(N_CORES)), trace=_trace, tmpdir=_tmpdir
    )
    out = np.empty((B, S, DV), np.float32)
    for core in range(N_CORES):
        b, h = core // 2, core % 2
        r = res.results[core]
        out[b, h * SQ:(h + 1) * SQ, :] = (r["outT"] / r["sums"]).T
    if _trace:
        kernel.last_results = res
    return out
